# revision 39
# baseline (speedup 1.0000x reference)
"""Tensor-parallel fused attention (QKV + RoPE + causal SDPA + out-proj) for
one TRN2 chip (8 NeuronCores), written in Bass/Tile.

Sharding: each core owns H/8 = 2 heads through QKV+RoPE+SDPA. The head
outputs are AllGathered (bf16, per batch+q-chunk) and the output projection
is sharded by OUTPUT columns (each core computes out[:, c*256:(c+1)*256]),
so the only collective is a cheap AllGather instead of an AllReduce. The
host assembles the full output by concatenating the 8 column slices.

Compute structure (all bf16 matmuls, fp32 PSUM):
- Attention runs in the scores-transposed orientation S^T[k, q], so the
  A@V matmul needs no probability transpose; V is produced directly in
  [tok, d] layout by running the V-projection with the x tile as the
  stationary operand (no PE transposes anywhere).
- Softmax runs without max-subtraction (max |logit| ~ 5.8, far from
  overflow). The denominator is accumulated on DVE in bf16 (all-bf16 DVE
  ops run the 2x perf mode) and collapsed across partitions with an
  all-ones matmul, which also broadcasts it to all 128 partitions; the
  reciprocal then runs full-width on DVE.
- The causal mask is one precomputed wedge (l >= p), identical for every
  diagonal 128-tile, applied by DVE multiply.
- Out-proj runs in the out^T orientation (stationary = wout oc-slice,
  moving = 512 gathered tokens -> N=512 matmuls, half the instruction
  count); the host un-transposes, and the tail q-chunk lands in a
  separate [tok, oc] buffer.

Schedule (from trace analysis; the chip power-throttles PE to ~13/16
clock when all 8 cores run, so wall time ~= PE cycles/1.95GHz + idle):
- collective_compute BLOCKS its engine queue until the collective
  completes, so gpsimd hosts ONLY collectives (no affine_select or
  partition_broadcast in steady state); the warmup AllGather fires at
  t~0 to absorb the one-time collective-init barrier + launch skew.
- The first chunk's x/wqkv DMAs are kt-sliced so PE starts early; the
  rope half-swap rides the scalar engine's private DGE queue (on the
  shared sync queue it lands behind MB-scale loads, ~15us stalls).
- Score matmuls interleave the two heads per k-tile and the AV matmuls
  pipeline one k-tile behind, so the exp pipe (ACT) never paces PE;
  PSUM->SBUF copies alternate ACT/DVE to balance both engines.
- Each batch's first attention chunk is hoisted into its QKV phase
  (PE-pinned behind QKV chunk 2) so the ~15-25us AllGathers spread out;
  chunk 0 has dedicated q0/k0/v0 tiles so next-batch rope writes do not
  chain behind the whole previous batch's attention reads (the q/k/v
  rings are single-buffered).
- Out-proj drains from a FIFO kept ~4 AllGathers behind attention, each
  out-proj PE-pinned after the most recent attention matmul.
- The last batch processes big chunks descending and its first q-chunk
  with the ao output AllGathered in four 128-token grains, banking
  out-proj work so the tail drains PE-dense with ~10us exposure.
"""

import os
import sys
import numpy as np

for _p in ("/opt/trn_rl_repo",):
    if _p not in sys.path:
        sys.path.insert(0, _p)

import ml_dtypes

import concourse.bass as bass
import concourse.mybir as mybir
import concourse.tile as tile
from concourse import bacc
from concourse.bass_utils import run_bass_kernel_spmd
from concourse.tile_rust import add_dep_helper

BF16 = mybir.dt.bfloat16
F32 = mybir.dt.float32
P = 128          # head_dim == SBUF partitions
CH = 512         # token chunk (matmul moving N)
SUB = 128        # tail sub-chunk

# full-size problem constants
B_FULL, T_FULL, D_FULL = 4, 2048, 2048
H_FULL = 16
N_CORES = 8


def build_nc(B, T, D, H, n_cores):
    """Build the per-core SPMD Bass graph. Returns compiled Bacc."""
    HPC = H // n_cores            # heads per core
    KT = D // P                   # k-tiles of the QKV contraction
    NQC = T // CH                 # q-chunks per batch
    NKT = T // P                  # k-tiles per batch (attention)
    TOK = B * T
    NOUT = D // n_cores           # out-proj columns per core
    HT = H                        # f-tiles (128 rows each) in out-proj
    SM_SCALE = 1.0 / float(np.sqrt(P))
    TPC = CH // P                 # 128-token tiles per chunk
    NSUB = CH // SUB              # sub-chunks in the tail chunk

    nc = bacc.Bacc("TRN2", target_bir_lowering=False, debug=False,
                   num_devices=n_cores)

    xT = nc.dram_tensor("xT", [D, TOK], BF16, kind="ExternalInput")
    wqkv = nc.dram_tensor("wqkv", [D, 3 * HPC * P], BF16, kind="ExternalInput")
    wout = nc.dram_tensor("wout", [H * P, NOUT], BF16, kind="ExternalInput")
    ropec = nc.dram_tensor("ropec", [P, T], BF16, kind="ExternalInput")
    ropes = nc.dram_tensor("ropes", [P, T], BF16, kind="ExternalInput")
    # main output is TRANSPOSED [oc, tok] (out-proj runs in the out^T
    # orientation for N=512 matmuls); host un-transposes. The tail
    # q-chunk (last batch, tokens 0:CH) lands in outS in [tok, oc].
    out = nc.dram_tensor("out", [NOUT, TOK], F32, kind="ExternalOutput")
    outS = nc.dram_tensor("outS", [CH, NOUT], F32, kind="ExternalOutput")

    cc_in = [nc.dram_tensor(f"ccin{b}", [NQC, HPC * P, CH], BF16)
             for b in range(B)]
    cc_out = [nc.dram_tensor(f"ccout{b}", [NQC, H * P, CH], BF16,
                             addr_space="Shared") for b in range(B)]
    # tail sub-chunk collectives (last batch, q-chunk 0, 128-token grains)
    ccs_in = [nc.dram_tensor(f"ccsin{s}", [HPC * P, SUB], BF16)
              for s in range(NSUB)]
    ccs_out = [nc.dram_tensor(f"ccsout{s}", [H * P, SUB], BF16,
                              addr_space="Shared") for s in range(NSUB)]

    xT_r = xT.ap().rearrange("(kt p) n -> p kt n", p=P)
    wqkv_r = wqkv.ap().rearrange("(kt p) f -> p kt f", p=P)
    wout_r = wout.ap().rearrange("(ft p) n -> p ft n", p=P)

    with tile.TileContext(nc) as tc:
        from contextlib import ExitStack
        with ExitStack() as ctx:
            consts = ctx.enter_context(tc.tile_pool(name="consts", bufs=1))
            px = ctx.enter_context(tc.tile_pool(name="px", bufs=2))
            pqkv = ctx.enter_context(tc.tile_pool(name="pqkv", bufs=1))
            prope = ctx.enter_context(tc.tile_pool(name="prope", bufs=2))
            pexp = ctx.enter_context(tc.tile_pool(name="pexp", bufs=1))
            pden = ctx.enter_context(tc.tile_pool(name="pden", bufs=1))
            pv = ctx.enter_context(tc.tile_pool(name="pv", bufs=1))
            pao = ctx.enter_context(tc.tile_pool(name="pao", bufs=2))
            pop = ctx.enter_context(tc.tile_pool(name="pop", bufs=4))
            poo = ctx.enter_context(tc.tile_pool(name="poo", bufs=2))

            pp_qkv = ctx.enter_context(
                tc.tile_pool(name="pp_qkv", bufs=2, space="PSUM"))
            pp_sc = ctx.enter_context(
                tc.tile_pool(name="pp_sc", bufs=2, space="PSUM"))
            pp_av = ctx.enter_context(
                tc.tile_pool(name="pp_av", bufs=2, space="PSUM"))
            pp_den = ctx.enter_context(
                tc.tile_pool(name="pp_den", bufs=1, space="PSUM"))
            pp_op = ctx.enter_context(
                tc.tile_pool(name="pp_op", bufs=1, space="PSUM"))

            # --- resident constants (DMAs mostly deferred: the first
            # chunk's kt-sliced loads go first so PE starts ~1us in) ---
            wq_sb = consts.tile([P, KT, 3 * HPC * P], BF16)
            wo_sb = consts.tile([P, HT, NOUT], BF16)
            rc_sb = consts.tile([P, T], BF16)
            rs_sb = consts.tile([P, T], BF16)
            ones_sb = consts.tile([P, P], BF16)
            mask_sb = consts.tile([P, CH], BF16)

            # The warmup AllGather goes FIRST: its trigger starts the
            # one-time collective-init barrier (which also absorbs cross-
            # core launch skew) as early as possible. collective_compute
            # blocks the gpsimd queue until the collective completes, so
            # nothing else may live on gpsimd: the causal mask is a
            # precomputed wedge applied by DVE multiply (the wedge
            # l >= p is the same for every diagonal 128-tile).
            warm_in = nc.dram_tensor("warm_in", [P, 16], BF16)
            warm_out = nc.dram_tensor("warm_out", [P * n_cores, 16], BF16,
                                      addr_space="Shared")
            warm_sb = consts.tile([P, 16], BF16)
            nc.vector.memset(warm_sb[:], 0.0)
            nc.vector.memset(mask_sb[:], 1.0)
            nc.gpsimd.affine_select(
                out=mask_sb[:], in_=mask_sb[:],
                compare_op=mybir.AluOpType.is_ge,
                fill=0.0, base=0, channel_multiplier=-1,
                pattern=[[1, CH]])
            nc.sync.dma_start(out=warm_in.ap(), in_=warm_sb[:])
            nc.gpsimd.collective_compute(
                "AllGather", mybir.AluOpType.bypass,
                replica_groups=[list(range(n_cores))],
                ins=[warm_in.ap().opt()], outs=[warm_out.ap().opt()])

            def emit_qkv_chunk(b, cc, q_sb, k_sb, v_sb, q0_sb, k0_sb,
                               v0_sb, sliced=False):
                pos0 = cc * CH
                tok0 = b * T + pos0
                x_sb = px.tile([P, KT, CH], BF16, tag="x", name="x")
                if sliced:
                    # interleave x and wqkv kt-slices so matmul kt can
                    # start as soon as its two operand slices land; rope
                    # tables (needed ~4us in) follow the first two pairs
                    for kt in range(KT):
                        nc.sync.dma_start(out=x_sb[:, kt, :],
                                          in_=xT_r[:, kt, tok0:tok0 + CH])
                        # wqkv rides the scalar engine's DGE queue
                        # (empty at t=0) so x and weights stream through
                        # parallel DMA paths at startup
                        nc.scalar.dma_start(out=wq_sb[:, kt, :],
                                            in_=wqkv_r[:, kt, :])
                        if kt == 1:
                            nc.sync.dma_start(out=rc_sb[:, 0:CH],
                                              in_=ropec.ap()[:, 0:CH])
                            nc.sync.dma_start(out=rs_sb[:, 0:CH],
                                              in_=ropes.ap()[:, 0:CH])
                else:
                    for xh in range(2):
                        nc.sync.dma_start(
                            out=x_sb[:, xh * KT // 2:(xh + 1) * KT // 2, :],
                            in_=xT_r[:, xh * KT // 2:(xh + 1) * KT // 2,
                                     tok0:tok0 + CH])
                def rope_tail(fi, ps):
                    h = fi % HPC
                    if cc == 0:
                        dst = (q0_sb if fi < HPC else k0_sb)
                        dsl = dst[:, h, 0:CH]
                    else:
                        dst = (q_sb if fi < HPC else k_sb)
                        dsl = dst[:, h, pos0:pos0 + CH]
                    raw = prope.tile([P, CH], BF16, tag="raw", name="raw")
                    # alternate copy engine: ACT is exp-burst-loaded,
                    # DVE is den/mask-loaded - split the difference
                    if fi % 2 == 0:
                        nc.scalar.copy(raw[:], ps[:])
                    else:
                        nc.vector.tensor_copy(raw[:], ps[:])
                    sw = prope.tile([P, CH], BF16, tag="sw", name="sw")
                    half = P // 2
                    # swap rides the scalar engine's private DGE queue: on
                    # the shared sync queue it can land behind MB-scale
                    # loads and stall the whole rope chain ~15us
                    nc.scalar.dma_start(out=sw[0:half, :],
                                        in_=raw[half:P, :])
                    nc.scalar.dma_start(out=sw[half:P, :],
                                        in_=raw[0:half, :])
                    # bf16 intermediates: all-bf16 DVE ops run in the
                    # 2x perf mode (428ns vs 687ns per 512-wide op); q/k
                    # are bf16 downstream anyway
                    t1 = prope.tile([P, CH], BF16, tag="t1", name="t1")
                    t2 = prope.tile([P, CH], BF16, tag="t2", name="t2")
                    nc.vector.tensor_tensor(
                        t1[:], raw[:], rc_sb[:, pos0:pos0 + CH],
                        mybir.AluOpType.mult)
                    nc.vector.tensor_tensor(
                        t2[:], sw[:], rs_sb[:, pos0:pos0 + CH],
                        mybir.AluOpType.mult)
                    nc.vector.tensor_tensor(
                        dsl, t1[:], t2[:], mybir.AluOpType.add)

                # k heads FIRST: attention at the next batch boundary
                # needs this chunk's k/v before its q
                fi_order = list(range(HPC, 2 * HPC)) + list(range(HPC))
                for fi in fi_order:
                    ps = pp_qkv.tile([P, CH], F32, tag="qkv",
                                     name="qkvps")
                    for kt in range(KT):
                        nc.tensor.matmul(
                            ps[:],
                            wq_sb[:, kt, fi * P:(fi + 1) * P],
                            x_sb[:, kt, :],
                            start=(kt == 0), stop=(kt == KT - 1))
                    rope_tail(fi, ps)
                # v heads: project straight into [tok, d] layout
                # (x tile stationary, Wv moving) - no PE transpose needed
                for tt in range(TPC):
                    kt_g = cc * TPC + tt
                    psv = pp_qkv.tile([P, HPC * P], F32, tag="qkv",
                                      name="vps")
                    for kt in range(KT):
                        last_mm = nc.tensor.matmul(
                            psv[:],
                            x_sb[:, kt, tt * P:(tt + 1) * P],
                            wq_sb[:, kt, 2 * HPC * P:3 * HPC * P],
                            start=(kt == 0), stop=(kt == KT - 1))
                    for h in range(HPC):
                        vdst = (v0_sb[h][:, tt, :] if cc == 0
                                else v_sb[h][:, kt_g, :])
                        if h % 2 == 0:
                            nc.scalar.copy(vdst,
                                           psv[:, h * P:(h + 1) * P])
                        else:
                            nc.vector.tensor_copy(vdst,
                                                  psv[:, h * P:(h + 1) * P])
                return last_mm

            def emit_outproj(b, qcs, order_after=None, fast_dma=False):
                # out^T orientation: stationary = wout oc-slice, moving =
                # the full 512-token gathered activation -> N=512 matmuls
                # (half the instruction count of the [tok, oc] orientation)
                for cc in qcs:
                    a_sb = pop.tile([P, HT, CH], BF16, tag="opin", bufs=2)
                    src = cc_out[b].ap()[cc].rearrange(
                        "(ft p) t -> p ft t", p=P)
                    # split along ft so the first matmuls start after a
                    # quarter of the load (matters when the AllGather
                    # just finished and this DMA is on the critical path)
                    for fh in range(4):
                        # in the tail ~25MB funnels through the sync DGE
                        # queue; ops whose AllGather is long done can
                        # safely split across the scalar queue (a pending
                        # AG-wait there would block the exp stream)
                        eng = nc.scalar if (fast_dma and fh >= 2) else nc.sync
                        eng.dma_start(
                            out=a_sb[:, fh * HT // 4:(fh + 1) * HT // 4, :],
                            in_=src[:, fh * HT // 4:(fh + 1) * HT // 4, :])
                    tok0 = b * T + cc * CH
                    for oc in range(NOUT // P):
                        po = pp_op.tile([P, CH], F32, tag="op")
                        for ft in range(HT):
                            mm = nc.tensor.matmul(
                                po[:],
                                wo_sb[:, ft, oc * P:(oc + 1) * P],
                                a_sb[:, ft, :],
                                start=(ft == 0), stop=(ft == HT - 1))
                            if order_after is not None:
                                # keep these matmuls AFTER the newer
                                # attention work in the PE stream: the
                                # scheduler's cost model under-prices the
                                # AllGather and would otherwise stall PE
                                add_dep_helper(
                                    mm.ins, order_after.ins, sync=False,
                                    reason="outproj after attn PE order")
                                order_after = None
                        oo = poo.tile([P, CH], F32, tag="oo")
                        if oc % 2 == 0:
                            nc.scalar.copy(oo[:], po[:])
                        else:
                            nc.vector.tensor_copy(oo[:], po[:])
                        nc.sync.dma_start(
                            out=out.ap()[oc * P:(oc + 1) * P,
                                         tok0:tok0 + CH],
                            in_=oo[:])

            def emit_outproj_sub(s, order_after=None):
                a_sb = pop.tile([P, HT, SUB], BF16, tag="opins", bufs=2)
                src = ccs_out[s].ap().rearrange("(ft p) t -> p ft t", p=P)
                nc.sync.dma_start(out=a_sb[:, 0:HT // 2, :],
                                  in_=src[:, 0:HT // 2, :])
                nc.scalar.dma_start(out=a_sb[:, HT // 2:, :],
                                    in_=src[:, HT // 2:, :])
                po = pp_op.tile([P, NOUT], F32, tag="op")
                for ft in range(HT):
                    mm = nc.tensor.matmul(
                        po[:], a_sb[:, ft, :], wo_sb[:, ft, :],
                        start=(ft == 0), stop=(ft == HT - 1))
                    if order_after is not None:
                        add_dep_helper(
                            mm.ins, order_after.ins, sync=False,
                            reason="outproj after attn PE order")
                        order_after = None
                oo = poo.tile([P, NOUT], F32, tag="oos")
                nc.vector.tensor_copy(oo[:], po[:])
                nc.sync.dma_start(out=outS.ap()[s * SUB:(s + 1) * SUB, :],
                                  in_=oo[:])

            def emit_attn_chunk(b, qc, q_sb, k_sb, v_sb, order_pin=None):
                """One attention q-chunk (both heads) + its AllGather.
                Returns the last AV matmul for PE-order pinning."""
                nkt = (qc + 1) * CH // P
                q0 = qc * CH
                diag0 = qc * CH // P
                expt = [None] * HPC
                den = [None] * HPC
                av = [None] * HPC
                for h in range(HPC):
                    expt[h] = pexp.tile([P, NKT, CH], BF16,
                                        tag=f"e{h}", name=f"e{h}")
                    den[h] = pden.tile([P, CH], BF16,
                                       tag=f"den{h}", name=f"den{h}")
                    av[h] = pp_av.tile([P, CH], F32, tag="av", name="av")

                def av_mm(h, kt):
                    col0 = (kt - diag0) * P if kt >= diag0 else 0
                    vsrc = (v0_sb[h][:, kt, :] if kt < TPC
                            else v_sb[h][:, kt, :])
                    return nc.tensor.matmul(
                        av[h][:, col0:CH], vsrc,
                        expt[h][:, kt, col0:CH],
                        start=(kt == 0), stop=(kt == nkt - 1))

                # Interleave the two heads per k-tile AND pipeline the AV
                # matmuls one k-tile behind the score matmuls: the exp pipe
                # runs ~1.6x slower than a score matmul, so scores alone
                # would be ACT-paced; with AV interleaved PE stays the pacer
                for kt in range(nkt):
                    # columns qq < (kt-diag0)*P of a diagonal tile are
                    # fully masked: restrict all work to qq >= col0
                    col0 = (kt - diag0) * P if kt >= diag0 else 0
                    ncol = CH - col0
                    for h in range(HPC):
                        sc = pp_sc.tile([P, CH], F32, tag="sc", name="sc")
                        ksrc = (k0_sb[:, h, kt * P:(kt + 1) * P]
                                if kt < TPC
                                else k_sb[:, h, kt * P:(kt + 1) * P])
                        qsrc = (q0_sb[:, h, col0:CH] if qc == 0
                                else q_sb[:, h, q0 + col0:q0 + CH])
                        mm_sc = nc.tensor.matmul(
                            sc[:, col0:CH], ksrc, qsrc,
                            start=True, stop=True)
                        if order_pin is not None:
                            # keep hoisted attention behind this point in
                            # the PE stream so its rope (queued behind
                            # attention DVE bursts) is done in time
                            add_dep_helper(
                                mm_sc.ins, order_pin.ins, sync=False,
                                reason="hoisted attn after qkv PE order")
                            order_pin = None
                        es = expt[h][:, kt, col0:CH]
                        nc.scalar.activation(
                            es, sc[:, col0:CH],
                            mybir.ActivationFunctionType.Exp,
                            scale=SM_SCALE)
                        if kt >= diag0:
                            # causal within the restricted block: keep
                            # lower triangle (i >= kk) via mask multiply
                            nc.vector.tensor_tensor(
                                es, es, mask_sb[:, 0:ncol],
                                mybir.AluOpType.mult)
                        if kt == 0:
                            # kt=0 always covers the full width; initialize
                            # the accumulator by copy
                            nc.vector.tensor_copy(den[h][:], es)
                        else:
                            nc.vector.tensor_tensor(
                                den[h][:, col0:CH], den[h][:, col0:CH],
                                es, mybir.AluOpType.add)
                    if kt > 0:
                        for h in range(HPC):
                            av_mm(h, kt - 1)
                last_av = None
                for h in range(HPC):
                    last_av = av_mm(h, nkt - 1)
                for h in range(HPC):
                    dbc = pp_den.tile([P, CH], F32, tag="dbc", name="dbc")
                    nc.tensor.matmul(dbc[:], ones_sb[:], den[h][:],
                                     start=True, stop=True)
                    # the ones-matmul already broadcast the denominator to
                    # all partitions: reciprocal runs full-width on DVE
                    recb = pden.tile([P, CH], F32, tag=f"rb{h}",
                                     name=f"rb{h}")
                    nc.vector.reciprocal_approx_fast(recb[:], dbc[:])
                    ao = pao.tile([P, CH], BF16, tag="ao", name="ao")
                    nc.vector.tensor_tensor(ao[:], av[h][:], recb[:],
                                            mybir.AluOpType.mult)
                    nc.sync.dma_start(
                        out=cc_in[b].ap()[qc, h * P:(h + 1) * P, :],
                        in_=ao[:])
                nc.gpsimd.collective_compute(
                    "AllGather", mybir.AluOpType.bypass,
                    replica_groups=[list(range(n_cores))],
                    ins=[cc_in[b].ap()[qc].opt()],
                    outs=[cc_out[b].ap()[qc].opt()])
                return last_av

            def emit_attn_sub(b, s, q_sb, k_sb, v_sb, order_pin=None):
                """128-token attention sub-chunk of q-chunk 0 (tail)."""
                nkt = s + 1
                q0 = s * SUB
                last_av = None
                expt = [None] * HPC
                den = [None] * HPC
                for h in range(HPC):
                    expt[h] = pexp.tile([P, NSUB, SUB], BF16,
                                        tag=f"se{h}", name=f"se{h}")
                    den[h] = pden.tile([P, SUB], BF16,
                                       tag=f"sden{h}", name=f"sden{h}")
                for kt in range(nkt):
                    for h in range(HPC):
                        sc = pp_sc.tile([P, SUB], F32, tag="sc", name="scs")
                        mm_sc = nc.tensor.matmul(
                            sc[:],
                            k_sb[:, h, kt * P:(kt + 1) * P],
                            q_sb[:, h, q0:q0 + SUB],
                            start=True, stop=True)
                        if order_pin is not None:
                            add_dep_helper(
                                mm_sc.ins, order_pin.ins, sync=False,
                                reason="hoisted attn after qkv PE order")
                            order_pin = None
                        es = expt[h][:, kt, :]
                        nc.scalar.activation(
                            es, sc[:],
                            mybir.ActivationFunctionType.Exp,
                            scale=SM_SCALE)
                        if kt == s:  # diagonal tile
                            nc.vector.tensor_tensor(
                                es, es, mask_sb[:, 0:SUB],
                                mybir.AluOpType.mult)
                        if kt == 0:
                            nc.vector.tensor_copy(den[h][:], es)
                        else:
                            nc.vector.tensor_tensor(
                                den[h][:], den[h][:], es,
                                mybir.AluOpType.add)
                for h in range(HPC):
                    av = pp_av.tile([P, SUB], F32, tag="av", name="avs")
                    for kt in range(nkt):
                        last_av = nc.tensor.matmul(
                            av[:], v_sb[h][:, kt, :], expt[h][:, kt, :],
                            start=(kt == 0), stop=(kt == nkt - 1))
                    dbc = pp_den.tile([P, SUB], F32, tag="dbc", name="dbcs")
                    nc.tensor.matmul(dbc[:], ones_sb[:], den[h][:],
                                     start=True, stop=True)
                    recb = pden.tile([P, SUB], F32, tag=f"srb{h}",
                                     name=f"srb{h}")
                    nc.vector.reciprocal_approx_fast(recb[:], dbc[:])
                    ao = pao.tile([P, SUB], BF16, tag="ao", name="aos")
                    nc.vector.tensor_tensor(ao[:], av[:], recb[:],
                                            mybir.AluOpType.mult)
                    nc.sync.dma_start(
                        out=ccs_in[s].ap()[h * P:(h + 1) * P, :],
                        in_=ao[:])
                nc.gpsimd.collective_compute(
                    "AllGather", mybir.AluOpType.bypass,
                    replica_groups=[list(range(n_cores))],
                    ins=[ccs_in[s].ap().opt()],
                    outs=[ccs_out[s].ap().opt()])
                return last_av

            def alloc_qkv_tiles():
                q_sb = pqkv.tile([P, HPC, T], BF16, tag="q", name="q")
                k_sb = pqkv.tile([P, HPC, T], BF16, tag="k", name="k")
                v_sb = [pv.tile([P, NKT, P], BF16, tag=f"v{h}", name=f"v{h}")
                        for h in range(HPC)]
                # chunk 0 gets its own small tiles: the single-buffered
                # q/k/v rings would otherwise chain next-batch rope
                # writes behind ALL of this batch's attention reads,
                # stalling the hoisted first-chunk attention ~3us/batch
                q0_sb = pqkv.tile([P, HPC, CH], BF16, tag="q0", name="q0")
                k0_sb = pqkv.tile([P, HPC, CH], BF16, tag="k0", name="k0")
                v0_sb = [pv.tile([P, TPC, P], BF16, tag=f"v0{h}",
                                 name=f"v0{h}") for h in range(HPC)]
                return q_sb, k_sb, v_sb, q0_sb, k0_sb, v0_sb

            # Phase-separated per batch: all QKV chunks, then attention
            # (full interleaving measured slower: rope-write/attention-read
            # WAR ping-pong on q_sb/k_sb). One exception: the NEXT batch's
            # first QKV chunk is hoisted before this batch's LAST attention
            # chunk - its matmuls and psum->SBUF copies are WAR-free, so
            # they fill the exp-pipeline bubble at the batch boundary.
            tiles = alloc_qkv_tiles()
            emit_qkv_chunk(0, 0, *tiles, sliced=True)
            # remaining deferred const loads, behind chunk 0's slices
            nc.sync.dma_start(out=rc_sb[:, CH:], in_=ropec.ap()[:, CH:])
            nc.sync.dma_start(out=rs_sb[:, CH:], in_=ropes.ap()[:, CH:])
            nc.vector.memset(ones_sb[:], 1.0)
            nc.sync.dma_start(out=wo_sb[:], in_=wout_r)

            # out-proj FIFO: drained ~4 AllGathers behind attention so the
            # 15-25us collectives never gate the in-order PE queue; each
            # out-proj is pinned after the most recent attention matmul
            agq = []

            def drain_op(upto, order_after, tail=False):
                while len(agq) > upto:
                    item = agq.pop(0)
                    if item[0] == "c":
                        emit_outproj(item[1], [item[2]],
                                     order_after=order_after,
                                     fast_dma=(tail and item[1] < B - 1))
                    else:
                        emit_outproj_sub(item[1], order_after=order_after)

            # prologue: rest of QKV(0) with attn(0,0) hoisted in so its
            # AllGather issues ~50us early and the CC queue never bunches
            emit_qkv_chunk(0, 1, *tiles)
            pin0 = emit_qkv_chunk(0, 2, *tiles)
            last_av = emit_attn_chunk(0, 0, *tiles, order_pin=pin0)
            agq.append(("c", 0, 0))
            emit_qkv_chunk(0, 3, *tiles)

            for b in range(B - 1):
                tiles_next = alloc_qkv_tiles()
                last_av = emit_attn_chunk(b, 1, *tiles)
                agq.append(("c", b, 1))
                drain_op(4, last_av)
                last_av = emit_attn_chunk(b, 2, *tiles)
                agq.append(("c", b, 2))
                drain_op(4, last_av)
                emit_qkv_chunk(b + 1, 0, *tiles_next)
                last_av = emit_attn_chunk(b, 3, *tiles)
                agq.append(("c", b, 3))
                drain_op(4, last_av)
                emit_qkv_chunk(b + 1, 1, *tiles_next)
                pin = emit_qkv_chunk(b + 1, 2, *tiles_next)
                # hoist next batch's first attention (it only needs QKV
                # chunk 0) into its QKV phase, PE-pinned behind QKV
                # chunk 2 so the scheduler cannot run it before the rope
                # DVE chain (queued behind attention DVE bursts) is done
                if b + 1 < B - 1:
                    last_av = emit_attn_chunk(b + 1, 0, *tiles_next,
                                              order_pin=pin)
                    agq.append(("c", b + 1, 0))
                    drain_op(4, last_av)
                else:
                    # tail's 128-token sub-chunks (q-chunk 0 of the last
                    # batch, also QKV-chunk-0-only): their small
                    # AllGathers complete during QKV(B-1), and NOT
                    # draining here banks out-proj work for the tail
                    for s in range(NSUB - 1, -1, -1):
                        last_av = emit_attn_sub(b + 1, s, *tiles_next,
                                                order_pin=pin)
                        agq.append(("s", s))
                        pin = None
                emit_qkv_chunk(b + 1, 3, *tiles_next)
                tiles = tiles_next

            # last batch: big chunks in DESCENDING size order (their
            # AllGathers start as early as possible) interleaved with the
            # banked out-proj backlog; every tail AllGather is already
            # done or in flight when its out-proj comes up, so the tail
            # drains PE-dense
            for qc in range(NQC - 1, 0, -1):
                last_av = emit_attn_chunk(B - 1, qc, *tiles)
                agq.append(("c", B - 1, qc))
                drain_op(8, last_av, tail=True)
            drain_op(0, last_av, tail=True)

    nc.compile()
    return nc


def shard_inputs(x, rope_cos, rope_sin, W_qkv, W_out, n_cores):
    """Host-side prep: transpose x, build rope tables in [d, pos] layout with
    the rotation sign folded in, slice per-core weight shards, cast to bf16."""
    B, T, D = x.shape
    H = W_qkv.shape[1] // (3 * P)
    HPC = H // n_cores
    NOUT = W_out.shape[1] // n_cores
    bf = ml_dtypes.bfloat16

    xT = np.ascontiguousarray(x.reshape(B * T, D).T).astype(bf)
    cosT = np.ascontiguousarray(rope_cos.T).astype(bf)          # [P, T]
    sinT = rope_sin.T.copy()
    sinT[:P // 2] = -sinT[:P // 2]
    sinT = np.ascontiguousarray(sinT).astype(bf)

    Wq3 = W_qkv.reshape(D, 3, H, P)  # [D, qkv, head, d]
    in_maps = []
    for c in range(n_cores):
        heads = range(c * HPC, (c + 1) * HPC)
        cols = [Wq3[:, i, h, :] for i in range(3) for h in heads]
        wqkv_c = np.ascontiguousarray(
            np.concatenate(cols, axis=1)).astype(bf)            # [D, 3*HPC*P]
        wout_c = np.ascontiguousarray(
            W_out[:, c * NOUT:(c + 1) * NOUT]).astype(bf)
        in_maps.append({
            "xT": xT, "wqkv": wqkv_c, "wout": wout_c,
            "ropec": cosT, "ropes": sinT,
        })
    return in_maps


def assemble_output(results, B, T, D, n_cores):
    NOUT = D // n_cores
    out = np.empty((B * T, D), np.float32)
    for c in range(n_cores):
        # device emits the transposed projection [oc, tok]; the last
        # batch's first q-chunk comes separately in [tok, oc]
        out[:, c * NOUT:(c + 1) * NOUT] = results[c]["out"].T
        out[(B - 1) * T:(B - 1) * T + CH, c * NOUT:(c + 1) * NOUT] = \
            results[c]["outS"]
    return out.reshape(B, T, D)


_NC_CACHE = {}


def _get_nc(B, T, D, H, n_cores):
    key = (B, T, D, H, n_cores)
    if key not in _NC_CACHE:
        _NC_CACHE[key] = build_nc(B, T, D, H, n_cores)
    return _NC_CACHE[key]


def run(x, rope_cos, rope_sin, W_qkv, W_out, trace=False):
    B, T, D = x.shape
    H = W_qkv.shape[1] // (3 * P)
    n_cores = N_CORES
    nc = _get_nc(B, T, D, H, n_cores)
    in_maps = shard_inputs(np.asarray(x, np.float32),
                           np.asarray(rope_cos, np.float32),
                           np.asarray(rope_sin, np.float32),
                           np.asarray(W_qkv, np.float32),
                           np.asarray(W_out, np.float32), n_cores)
    res = run_bass_kernel_spmd(nc, in_maps, core_ids=list(range(n_cores)),
                               trace=trace)
    out = assemble_output(res.results, B, T, D, n_cores)
    return out, res


def kernel(x, rope_cos, rope_sin, W_qkv, W_out):
    out, _ = run(x, rope_cos, rope_sin, W_qkv, W_out, trace=False)
    return out


# revision 40
# speedup vs baseline: 1.0356x; 1.0356x over previous
"""Tensor-parallel fused attention (QKV + RoPE + causal SDPA + out-proj) for
one TRN2 chip (8 NeuronCores), written in Bass/Tile.

Sharding: each core owns H/8 = 2 heads through QKV+RoPE+SDPA. The head
outputs are AllGathered (bf16, per batch+q-chunk) and the output projection
is sharded by OUTPUT columns (each core computes out[:, c*256:(c+1)*256]),
so the only collective is a cheap AllGather instead of an AllReduce. The
host assembles the full output by concatenating the 8 column slices.

Compute structure (all bf16 matmuls, fp32 PSUM):
- Attention runs in the scores-transposed orientation S^T[k, q], so the
  A@V matmul needs no probability transpose; V is produced directly in
  [tok, d] layout by running the V-projection with the x tile as the
  stationary operand (no PE transposes anywhere).
- Softmax runs without max-subtraction (max |logit| ~ 5.8, far from
  overflow). The denominator is accumulated on DVE in bf16 (all-bf16 DVE
  ops run the 2x perf mode) and collapsed across partitions with an
  all-ones matmul, which also broadcasts it to all 128 partitions; the
  reciprocal then runs full-width on DVE.
- The causal mask is one precomputed wedge (l >= p), identical for every
  diagonal 128-tile, applied by DVE multiply.
- Out-proj runs in the out^T orientation (stationary = wout oc-slice,
  moving = 512 gathered tokens -> N=512 matmuls, half the instruction
  count); the host un-transposes, and the tail q-chunk lands in a
  separate [tok, oc] buffer.

Schedule (from trace analysis; the chip power-throttles PE to ~13/16
clock when all 8 cores run, so wall time ~= PE cycles/1.95GHz + idle):
- collective_compute BLOCKS its engine queue until the collective
  completes, so gpsimd hosts ONLY collectives (no affine_select or
  partition_broadcast in steady state); the warmup AllGather fires at
  t~0 to absorb the one-time collective-init barrier + launch skew.
- The first chunk's x/wqkv DMAs are kt-sliced so PE starts early; the
  rope half-swap rides the scalar engine's private DGE queue (on the
  shared sync queue it lands behind MB-scale loads, ~15us stalls).
- Score matmuls interleave the two heads per k-tile and the AV matmuls
  pipeline one k-tile behind, so the exp pipe (ACT) never paces PE;
  PSUM->SBUF copies alternate ACT/DVE to balance both engines.
- Each batch's first attention chunk is hoisted into its QKV phase
  (PE-pinned behind QKV chunk 2) so the ~15-25us AllGathers spread out;
  chunk 0 has dedicated q0/k0/v0 tiles so next-batch rope writes do not
  chain behind the whole previous batch's attention reads (the q/k/v
  rings are single-buffered).
- Out-proj drains from a FIFO kept ~4 AllGathers behind attention, each
  out-proj PE-pinned after the most recent attention matmul.
- The last batch processes big chunks descending and its first q-chunk
  with the ao output AllGathered in four 128-token grains, banking
  out-proj work so the tail drains PE-dense with ~10us exposure.
"""

import os
import sys
import numpy as np

for _p in ("/opt/trn_rl_repo",):
    if _p not in sys.path:
        sys.path.insert(0, _p)

import ml_dtypes

import concourse.bass as bass
import concourse.mybir as mybir
import concourse.tile as tile
from concourse import bacc
from concourse.bass_utils import run_bass_kernel_spmd
from concourse.tile_rust import add_dep_helper

BF16 = mybir.dt.bfloat16
F32 = mybir.dt.float32
P = 128          # head_dim == SBUF partitions
CH = 512         # token chunk (matmul moving N)
SUB = 128        # tail sub-chunk

# full-size problem constants
B_FULL, T_FULL, D_FULL = 4, 2048, 2048
H_FULL = 16
N_CORES = 8


def build_nc(B, T, D, H, n_cores):
    """Build the per-core SPMD Bass graph. Returns compiled Bacc."""
    HPC = H // n_cores            # heads per core
    KT = D // P                   # k-tiles of the QKV contraction
    NQC = T // CH                 # q-chunks per batch
    NKT = T // P                  # k-tiles per batch (attention)
    TOK = B * T
    NOUT = D // n_cores           # out-proj columns per core
    HT = H                        # f-tiles (128 rows each) in out-proj
    SM_SCALE = 1.0 / float(np.sqrt(P))
    TPC = CH // P                 # 128-token tiles per chunk
    NSUB = CH // SUB              # sub-chunks in the tail chunk

    nc = bacc.Bacc("TRN2", target_bir_lowering=False, debug=False,
                   num_devices=n_cores)

    xT = nc.dram_tensor("xT", [D, TOK], BF16, kind="ExternalInput")
    wqkv = nc.dram_tensor("wqkv", [D, 3 * HPC * P], BF16, kind="ExternalInput")
    wout = nc.dram_tensor("wout", [H * P, NOUT], BF16, kind="ExternalInput")
    ropec = nc.dram_tensor("ropec", [P, T], BF16, kind="ExternalInput")
    ropes = nc.dram_tensor("ropes", [P, T], BF16, kind="ExternalInput")
    # main output is TRANSPOSED [oc, tok] (out-proj runs in the out^T
    # orientation for N=512 matmuls); host un-transposes. The tail
    # q-chunk (last batch, tokens 0:CH) lands in outS in [tok, oc].
    out = nc.dram_tensor("out", [NOUT, TOK], F32, kind="ExternalOutput")
    outS = nc.dram_tensor("outS", [CH, NOUT], F32, kind="ExternalOutput")

    cc_in = [nc.dram_tensor(f"ccin{b}", [NQC, HPC * P, CH], BF16)
             for b in range(B)]
    cc_out = [nc.dram_tensor(f"ccout{b}", [NQC, H * P, CH], BF16,
                             addr_space="Shared") for b in range(B)]
    # tail sub-chunk collectives (last batch, q-chunk 0, 128-token grains)
    ccs_in = [nc.dram_tensor(f"ccsin{s}", [HPC * P, SUB], BF16)
              for s in range(NSUB)]
    ccs_out = [nc.dram_tensor(f"ccsout{s}", [H * P, SUB], BF16,
                              addr_space="Shared") for s in range(NSUB)]

    xT_r = xT.ap().rearrange("(kt p) n -> p kt n", p=P)
    wqkv_r = wqkv.ap().rearrange("(kt p) f -> p kt f", p=P)
    wout_r = wout.ap().rearrange("(ft p) n -> p ft n", p=P)

    with tile.TileContext(nc) as tc:
        from contextlib import ExitStack
        with ExitStack() as ctx:
            consts = ctx.enter_context(tc.tile_pool(name="consts", bufs=1))
            px = ctx.enter_context(tc.tile_pool(name="px", bufs=2))
            pqkv = ctx.enter_context(tc.tile_pool(name="pqkv", bufs=1))
            prope = ctx.enter_context(tc.tile_pool(name="prope", bufs=2))
            pexp = ctx.enter_context(tc.tile_pool(name="pexp", bufs=1))
            pden = ctx.enter_context(tc.tile_pool(name="pden", bufs=1))
            pv = ctx.enter_context(tc.tile_pool(name="pv", bufs=1))
            pao = ctx.enter_context(tc.tile_pool(name="pao", bufs=2))
            pop = ctx.enter_context(tc.tile_pool(name="pop", bufs=4))
            poo = ctx.enter_context(tc.tile_pool(name="poo", bufs=2))

            pp_qkv = ctx.enter_context(
                tc.tile_pool(name="pp_qkv", bufs=2, space="PSUM"))
            pp_sc = ctx.enter_context(
                tc.tile_pool(name="pp_sc", bufs=2, space="PSUM"))
            pp_av = ctx.enter_context(
                tc.tile_pool(name="pp_av", bufs=2, space="PSUM"))
            pp_den = ctx.enter_context(
                tc.tile_pool(name="pp_den", bufs=1, space="PSUM"))
            pp_op = ctx.enter_context(
                tc.tile_pool(name="pp_op", bufs=1, space="PSUM"))

            # --- resident constants (DMAs mostly deferred: the first
            # chunk's kt-sliced loads go first so PE starts ~1us in) ---
            wq_sb = consts.tile([P, KT, 3 * HPC * P], BF16)
            wo_sb = consts.tile([P, HT, NOUT], BF16)
            rc_sb = consts.tile([P, T], BF16)
            rs_sb = consts.tile([P, T], BF16)
            ones_sb = consts.tile([P, P], BF16)
            mask_sb = consts.tile([P, CH], BF16)

            # The warmup AllGather goes FIRST: its trigger starts the
            # one-time collective-init barrier (which also absorbs cross-
            # core launch skew) as early as possible. collective_compute
            # blocks the gpsimd queue until the collective completes, so
            # nothing else may live on gpsimd: the causal mask is a
            # precomputed wedge applied by DVE multiply (the wedge
            # l >= p is the same for every diagonal 128-tile).
            warm_in = nc.dram_tensor("warm_in", [P, 16], BF16)
            warm_out = nc.dram_tensor("warm_out", [P * n_cores, 16], BF16,
                                      addr_space="Shared")
            warm_sb = consts.tile([P, 16], BF16)
            nc.vector.memset(warm_sb[:], 0.0)
            nc.vector.memset(mask_sb[:], 1.0)
            nc.gpsimd.affine_select(
                out=mask_sb[:], in_=mask_sb[:],
                compare_op=mybir.AluOpType.is_ge,
                fill=0.0, base=0, channel_multiplier=-1,
                pattern=[[1, CH]])
            nc.sync.dma_start(out=warm_in.ap(), in_=warm_sb[:])
            nc.gpsimd.collective_compute(
                "AllGather", mybir.AluOpType.bypass,
                replica_groups=[list(range(n_cores))],
                ins=[warm_in.ap().opt()], outs=[warm_out.ap().opt()])

            def emit_qkv_chunk(b, cc, q_sb, k_sb, v_sb, q0_sb, k0_sb,
                               v0_sb, sliced=False):
                pos0 = cc * CH
                tok0 = b * T + pos0
                x_sb = px.tile([P, KT, CH], BF16, tag="x", name="x")
                if sliced:
                    # interleave x and wqkv kt-slices so matmul kt can
                    # start as soon as its two operand slices land; rope
                    # tables (needed ~4us in) follow the first two pairs
                    for kt in range(KT):
                        nc.sync.dma_start(out=x_sb[:, kt, :],
                                          in_=xT_r[:, kt, tok0:tok0 + CH])
                        # wqkv rides the scalar engine's DGE queue
                        # (empty at t=0) so x and weights stream through
                        # parallel DMA paths at startup
                        nc.scalar.dma_start(out=wq_sb[:, kt, :],
                                            in_=wqkv_r[:, kt, :])
                        if kt == 1:
                            nc.sync.dma_start(out=rc_sb[:, 0:CH],
                                              in_=ropec.ap()[:, 0:CH])
                            nc.sync.dma_start(out=rs_sb[:, 0:CH],
                                              in_=ropes.ap()[:, 0:CH])
                else:
                    for xh in range(2):
                        nc.sync.dma_start(
                            out=x_sb[:, xh * KT // 2:(xh + 1) * KT // 2, :],
                            in_=xT_r[:, xh * KT // 2:(xh + 1) * KT // 2,
                                     tok0:tok0 + CH])
                def rope_tail(fi, ps):
                    h = fi % HPC
                    if cc == 0:
                        dst = (q0_sb if fi < HPC else k0_sb)
                        dsl = dst[:, h, 0:CH]
                    else:
                        dst = (q_sb if fi < HPC else k_sb)
                        dsl = dst[:, h, pos0:pos0 + CH]
                    raw = prope.tile([P, CH], BF16, tag="raw", name="raw")
                    # alternate copy engine: ACT is exp-burst-loaded,
                    # DVE is den/mask-loaded - split the difference
                    if fi % 2 == 0:
                        nc.scalar.copy(raw[:], ps[:])
                    else:
                        nc.vector.tensor_copy(raw[:], ps[:])
                    sw = prope.tile([P, CH], BF16, tag="sw", name="sw")
                    half = P // 2
                    # swap rides the scalar engine's private DGE queue: on
                    # the shared sync queue it can land behind MB-scale
                    # loads and stall the whole rope chain ~15us
                    nc.scalar.dma_start(out=sw[0:half, :],
                                        in_=raw[half:P, :])
                    nc.scalar.dma_start(out=sw[half:P, :],
                                        in_=raw[0:half, :])
                    # bf16 intermediates: all-bf16 DVE ops run in the
                    # 2x perf mode (428ns vs 687ns per 512-wide op); q/k
                    # are bf16 downstream anyway
                    t1 = prope.tile([P, CH], BF16, tag="t1", name="t1")
                    t2 = prope.tile([P, CH], BF16, tag="t2", name="t2")
                    nc.vector.tensor_tensor(
                        t1[:], raw[:], rc_sb[:, pos0:pos0 + CH],
                        mybir.AluOpType.mult)
                    nc.vector.tensor_tensor(
                        t2[:], sw[:], rs_sb[:, pos0:pos0 + CH],
                        mybir.AluOpType.mult)
                    nc.vector.tensor_tensor(
                        dsl, t1[:], t2[:], mybir.AluOpType.add)

                # k heads FIRST: attention at the next batch boundary
                # needs this chunk's k/v before its q
                fi_order = list(range(HPC, 2 * HPC)) + list(range(HPC))
                for fi in fi_order:
                    ps = pp_qkv.tile([P, CH], F32, tag="qkv",
                                     name="qkvps")
                    for kt in range(KT):
                        nc.tensor.matmul(
                            ps[:],
                            wq_sb[:, kt, fi * P:(fi + 1) * P],
                            x_sb[:, kt, :],
                            start=(kt == 0), stop=(kt == KT - 1))
                    rope_tail(fi, ps)
                # v heads: project straight into [tok, d] layout
                # (x tile stationary, Wv moving) - no PE transpose needed
                for tt in range(TPC):
                    kt_g = cc * TPC + tt
                    psv = pp_qkv.tile([P, HPC * P], F32, tag="qkv",
                                      name="vps")
                    for kt in range(KT):
                        last_mm = nc.tensor.matmul(
                            psv[:],
                            x_sb[:, kt, tt * P:(tt + 1) * P],
                            wq_sb[:, kt, 2 * HPC * P:3 * HPC * P],
                            start=(kt == 0), stop=(kt == KT - 1))
                    for h in range(HPC):
                        vdst = (v0_sb[h][:, tt, :] if cc == 0
                                else v_sb[h][:, kt_g, :])
                        if h % 2 == 0:
                            nc.scalar.copy(vdst,
                                           psv[:, h * P:(h + 1) * P])
                        else:
                            nc.vector.tensor_copy(vdst,
                                                  psv[:, h * P:(h + 1) * P])
                return last_mm

            def emit_outproj(b, qcs, order_after=None, fast_dma=False):
                # out^T orientation: stationary = wout oc-slice, moving =
                # the full 512-token gathered activation -> N=512 matmuls
                # (half the instruction count of the [tok, oc] orientation)
                for cc in qcs:
                    a_sb = pop.tile([P, HT, CH], BF16, tag="opin", bufs=2)
                    src = cc_out[b].ap()[cc].rearrange(
                        "(ft p) t -> p ft t", p=P)
                    # split along ft so the first matmuls start after a
                    # quarter of the load (matters when the AllGather
                    # just finished and this DMA is on the critical path)
                    for fh in range(4):
                        # in the tail ~25MB funnels through the sync DGE
                        # queue; ops whose AllGather is long done can
                        # safely split across the scalar queue (a pending
                        # AG-wait there would block the exp stream)
                        eng = nc.scalar if (fast_dma and fh >= 2) else nc.sync
                        eng.dma_start(
                            out=a_sb[:, fh * HT // 4:(fh + 1) * HT // 4, :],
                            in_=src[:, fh * HT // 4:(fh + 1) * HT // 4, :])
                    tok0 = b * T + cc * CH
                    for oc in range(NOUT // P):
                        po = pp_op.tile([P, CH], F32, tag="op")
                        for ft in range(HT):
                            mm = nc.tensor.matmul(
                                po[:],
                                wo_sb[:, ft, oc * P:(oc + 1) * P],
                                a_sb[:, ft, :],
                                start=(ft == 0), stop=(ft == HT - 1))
                            if order_after is not None:
                                # keep these matmuls AFTER the newer
                                # attention work in the PE stream: the
                                # scheduler's cost model under-prices the
                                # AllGather and would otherwise stall PE
                                add_dep_helper(
                                    mm.ins, order_after.ins, sync=False,
                                    reason="outproj after attn PE order")
                                order_after = None
                        oo = poo.tile([P, CH], F32, tag="oo")
                        if oc % 2 == 0:
                            nc.scalar.copy(oo[:], po[:])
                        else:
                            nc.vector.tensor_copy(oo[:], po[:])
                        nc.sync.dma_start(
                            out=out.ap()[oc * P:(oc + 1) * P,
                                         tok0:tok0 + CH],
                            in_=oo[:])

            def emit_outproj_sub(s, order_after=None):
                a_sb = pop.tile([P, HT, SUB], BF16, tag="opins", bufs=2)
                src = ccs_out[s].ap().rearrange("(ft p) t -> p ft t", p=P)
                nc.sync.dma_start(out=a_sb[:, 0:HT // 2, :],
                                  in_=src[:, 0:HT // 2, :])
                nc.scalar.dma_start(out=a_sb[:, HT // 2:, :],
                                    in_=src[:, HT // 2:, :])
                po = pp_op.tile([P, NOUT], F32, tag="op")
                for ft in range(HT):
                    mm = nc.tensor.matmul(
                        po[:], a_sb[:, ft, :], wo_sb[:, ft, :],
                        start=(ft == 0), stop=(ft == HT - 1))
                    if order_after is not None:
                        add_dep_helper(
                            mm.ins, order_after.ins, sync=False,
                            reason="outproj after attn PE order")
                        order_after = None
                oo = poo.tile([P, NOUT], F32, tag="oos")
                nc.vector.tensor_copy(oo[:], po[:])
                nc.sync.dma_start(out=outS.ap()[s * SUB:(s + 1) * SUB, :],
                                  in_=oo[:])

            def emit_attn_chunk(b, qc, q_sb, k_sb, v_sb, order_pin=None):
                """One attention q-chunk (both heads) + its AllGather.
                Returns the last AV matmul for PE-order pinning."""
                nkt = (qc + 1) * CH // P
                q0 = qc * CH
                diag0 = qc * CH // P
                expt = [None] * HPC
                den = [None] * HPC
                av = [None] * HPC
                for h in range(HPC):
                    expt[h] = pexp.tile([P, NKT, CH], BF16,
                                        tag=f"e{h}", name=f"e{h}")
                    den[h] = pden.tile([P, CH], BF16,
                                       tag=f"den{h}", name=f"den{h}")
                    av[h] = pp_av.tile([P, CH], F32, tag="av", name="av")

                def av_mm(h, kt):
                    col0 = (kt - diag0) * P if kt >= diag0 else 0
                    vsrc = (v0_sb[h][:, kt, :] if kt < TPC
                            else v_sb[h][:, kt, :])
                    return nc.tensor.matmul(
                        av[h][:, col0:CH], vsrc,
                        expt[h][:, kt, col0:CH],
                        start=(kt == 0), stop=(kt == nkt - 1))

                # Interleave the two heads per k-tile AND pipeline the AV
                # matmuls one k-tile behind the score matmuls: the exp pipe
                # runs ~1.6x slower than a score matmul, so scores alone
                # would be ACT-paced; with AV interleaved PE stays the pacer
                for kt in range(nkt):
                    # columns qq < (kt-diag0)*P of a diagonal tile are
                    # fully masked: restrict all work to qq >= col0
                    col0 = (kt - diag0) * P if kt >= diag0 else 0
                    ncol = CH - col0
                    for h in range(HPC):
                        sc = pp_sc.tile([P, CH], F32, tag="sc", name="sc")
                        ksrc = (k0_sb[:, h, kt * P:(kt + 1) * P]
                                if kt < TPC
                                else k_sb[:, h, kt * P:(kt + 1) * P])
                        qsrc = (q0_sb[:, h, col0:CH] if qc == 0
                                else q_sb[:, h, q0 + col0:q0 + CH])
                        mm_sc = nc.tensor.matmul(
                            sc[:, col0:CH], ksrc, qsrc,
                            start=True, stop=True)
                        if order_pin is not None:
                            # keep hoisted attention behind this point in
                            # the PE stream so its rope (queued behind
                            # attention DVE bursts) is done in time
                            add_dep_helper(
                                mm_sc.ins, order_pin.ins, sync=False,
                                reason="hoisted attn after qkv PE order")
                            order_pin = None
                        es = expt[h][:, kt, col0:CH]
                        nc.scalar.activation(
                            es, sc[:, col0:CH],
                            mybir.ActivationFunctionType.Exp,
                            scale=SM_SCALE)
                        if kt >= diag0:
                            # causal within the restricted block: keep
                            # lower triangle (i >= kk) via mask multiply
                            nc.vector.tensor_tensor(
                                es, es, mask_sb[:, 0:ncol],
                                mybir.AluOpType.mult)
                        if kt == 0:
                            # kt=0 always covers the full width; initialize
                            # the accumulator by copy
                            nc.vector.tensor_copy(den[h][:], es)
                        else:
                            nc.vector.tensor_tensor(
                                den[h][:, col0:CH], den[h][:, col0:CH],
                                es, mybir.AluOpType.add)
                    # AV runs TWO k-tiles behind the scores: the
                    # exp->mask chain lags ~1.7us; one tile of PE cover
                    # (524ns) left ~1us AV-wait gaps through every chunk
                    if kt > 1:
                        for h in range(HPC):
                            av_mm(h, kt - 2)
                last_av = None
                for klast in (nkt - 2, nkt - 1):
                    for h in range(HPC):
                        last_av = av_mm(h, klast)
                for h in range(HPC):
                    dbc = pp_den.tile([P, CH], F32, tag="dbc", name="dbc")
                    nc.tensor.matmul(dbc[:], ones_sb[:], den[h][:],
                                     start=True, stop=True)
                    # the ones-matmul already broadcast the denominator to
                    # all partitions: reciprocal runs full-width on DVE
                    recb = pden.tile([P, CH], F32, tag=f"rb{h}",
                                     name=f"rb{h}")
                    nc.vector.reciprocal_approx_fast(recb[:], dbc[:])
                    ao = pao.tile([P, CH], BF16, tag="ao", name="ao")
                    nc.vector.tensor_tensor(ao[:], av[h][:], recb[:],
                                            mybir.AluOpType.mult)
                    nc.sync.dma_start(
                        out=cc_in[b].ap()[qc, h * P:(h + 1) * P, :],
                        in_=ao[:])
                nc.gpsimd.collective_compute(
                    "AllGather", mybir.AluOpType.bypass,
                    replica_groups=[list(range(n_cores))],
                    ins=[cc_in[b].ap()[qc].opt()],
                    outs=[cc_out[b].ap()[qc].opt()])
                return last_av

            def emit_attn_sub(b, s, q_sb, k_sb, v_sb, order_pin=None):
                """128-token attention sub-chunk of q-chunk 0 (tail)."""
                nkt = s + 1
                q0 = s * SUB
                last_av = None
                expt = [None] * HPC
                den = [None] * HPC
                for h in range(HPC):
                    expt[h] = pexp.tile([P, NSUB, SUB], BF16,
                                        tag=f"se{h}", name=f"se{h}")
                    den[h] = pden.tile([P, SUB], BF16,
                                       tag=f"sden{h}", name=f"sden{h}")
                for kt in range(nkt):
                    for h in range(HPC):
                        sc = pp_sc.tile([P, SUB], F32, tag="sc", name="scs")
                        mm_sc = nc.tensor.matmul(
                            sc[:],
                            k_sb[:, h, kt * P:(kt + 1) * P],
                            q_sb[:, h, q0:q0 + SUB],
                            start=True, stop=True)
                        if order_pin is not None:
                            add_dep_helper(
                                mm_sc.ins, order_pin.ins, sync=False,
                                reason="hoisted attn after qkv PE order")
                            order_pin = None
                        es = expt[h][:, kt, :]
                        nc.scalar.activation(
                            es, sc[:],
                            mybir.ActivationFunctionType.Exp,
                            scale=SM_SCALE)
                        if kt == s:  # diagonal tile
                            nc.vector.tensor_tensor(
                                es, es, mask_sb[:, 0:SUB],
                                mybir.AluOpType.mult)
                        if kt == 0:
                            nc.vector.tensor_copy(den[h][:], es)
                        else:
                            nc.vector.tensor_tensor(
                                den[h][:], den[h][:], es,
                                mybir.AluOpType.add)
                for h in range(HPC):
                    av = pp_av.tile([P, SUB], F32, tag="av", name="avs")
                    for kt in range(nkt):
                        last_av = nc.tensor.matmul(
                            av[:], v_sb[h][:, kt, :], expt[h][:, kt, :],
                            start=(kt == 0), stop=(kt == nkt - 1))
                    dbc = pp_den.tile([P, SUB], F32, tag="dbc", name="dbcs")
                    nc.tensor.matmul(dbc[:], ones_sb[:], den[h][:],
                                     start=True, stop=True)
                    recb = pden.tile([P, SUB], F32, tag=f"srb{h}",
                                     name=f"srb{h}")
                    nc.vector.reciprocal_approx_fast(recb[:], dbc[:])
                    ao = pao.tile([P, SUB], BF16, tag="ao", name="aos")
                    nc.vector.tensor_tensor(ao[:], av[:], recb[:],
                                            mybir.AluOpType.mult)
                    nc.sync.dma_start(
                        out=ccs_in[s].ap()[h * P:(h + 1) * P, :],
                        in_=ao[:])
                nc.gpsimd.collective_compute(
                    "AllGather", mybir.AluOpType.bypass,
                    replica_groups=[list(range(n_cores))],
                    ins=[ccs_in[s].ap().opt()],
                    outs=[ccs_out[s].ap().opt()])
                return last_av

            def alloc_qkv_tiles():
                q_sb = pqkv.tile([P, HPC, T], BF16, tag="q", name="q")
                k_sb = pqkv.tile([P, HPC, T], BF16, tag="k", name="k")
                v_sb = [pv.tile([P, NKT, P], BF16, tag=f"v{h}", name=f"v{h}")
                        for h in range(HPC)]
                # chunk 0 gets its own small tiles: the single-buffered
                # q/k/v rings would otherwise chain next-batch rope
                # writes behind ALL of this batch's attention reads,
                # stalling the hoisted first-chunk attention ~3us/batch
                q0_sb = pqkv.tile([P, HPC, CH], BF16, tag="q0", name="q0")
                k0_sb = pqkv.tile([P, HPC, CH], BF16, tag="k0", name="k0")
                v0_sb = [pv.tile([P, TPC, P], BF16, tag=f"v0{h}",
                                 name=f"v0{h}") for h in range(HPC)]
                return q_sb, k_sb, v_sb, q0_sb, k0_sb, v0_sb

            # Phase-separated per batch: all QKV chunks, then attention
            # (full interleaving measured slower: rope-write/attention-read
            # WAR ping-pong on q_sb/k_sb). One exception: the NEXT batch's
            # first QKV chunk is hoisted before this batch's LAST attention
            # chunk - its matmuls and psum->SBUF copies are WAR-free, so
            # they fill the exp-pipeline bubble at the batch boundary.
            tiles = alloc_qkv_tiles()
            emit_qkv_chunk(0, 0, *tiles, sliced=True)
            # remaining deferred const loads, behind chunk 0's slices
            nc.sync.dma_start(out=rc_sb[:, CH:], in_=ropec.ap()[:, CH:])
            nc.sync.dma_start(out=rs_sb[:, CH:], in_=ropes.ap()[:, CH:])
            nc.vector.memset(ones_sb[:], 1.0)
            nc.sync.dma_start(out=wo_sb[:], in_=wout_r)

            # out-proj FIFO: drained ~4 AllGathers behind attention so the
            # 15-25us collectives never gate the in-order PE queue; each
            # out-proj is pinned after the most recent attention matmul
            agq = []

            def drain_op(upto, order_after, tail=False):
                while len(agq) > upto:
                    item = agq.pop(0)
                    if item[0] == "c":
                        emit_outproj(item[1], [item[2]],
                                     order_after=order_after,
                                     fast_dma=(tail and item[1] < B - 1))
                    else:
                        emit_outproj_sub(item[1], order_after=order_after)

            # prologue: rest of QKV(0) with attn(0,0) hoisted in so its
            # AllGather issues ~50us early and the CC queue never bunches
            emit_qkv_chunk(0, 1, *tiles)
            pin0 = emit_qkv_chunk(0, 2, *tiles)
            last_av = emit_attn_chunk(0, 0, *tiles, order_pin=pin0)
            agq.append(("c", 0, 0))
            emit_qkv_chunk(0, 3, *tiles)

            for b in range(B - 1):
                tiles_next = alloc_qkv_tiles()
                last_av = emit_attn_chunk(b, 1, *tiles)
                agq.append(("c", b, 1))
                drain_op(4, last_av)
                last_av = emit_attn_chunk(b, 2, *tiles)
                agq.append(("c", b, 2))
                drain_op(4, last_av)
                emit_qkv_chunk(b + 1, 0, *tiles_next)
                last_av = emit_attn_chunk(b, 3, *tiles)
                agq.append(("c", b, 3))
                drain_op(4, last_av)
                emit_qkv_chunk(b + 1, 1, *tiles_next)
                pin = emit_qkv_chunk(b + 1, 2, *tiles_next)
                # hoist next batch's first attention (it only needs QKV
                # chunk 0) into its QKV phase, PE-pinned behind QKV
                # chunk 2 so the scheduler cannot run it before the rope
                # DVE chain (queued behind attention DVE bursts) is done
                if b + 1 < B - 1:
                    last_av = emit_attn_chunk(b + 1, 0, *tiles_next,
                                              order_pin=pin)
                    agq.append(("c", b + 1, 0))
                    drain_op(4, last_av)
                else:
                    # tail's 128-token sub-chunks (q-chunk 0 of the last
                    # batch, also QKV-chunk-0-only): their small
                    # AllGathers complete during QKV(B-1), and NOT
                    # draining here banks out-proj work for the tail
                    for s in range(NSUB - 1, -1, -1):
                        last_av = emit_attn_sub(b + 1, s, *tiles_next,
                                                order_pin=pin)
                        agq.append(("s", s))
                        pin = None
                emit_qkv_chunk(b + 1, 3, *tiles_next)
                tiles = tiles_next

            # last batch: big chunks in DESCENDING size order (their
            # AllGathers start as early as possible) interleaved with the
            # banked out-proj backlog; every tail AllGather is already
            # done or in flight when its out-proj comes up, so the tail
            # drains PE-dense
            for qc in range(NQC - 1, 0, -1):
                last_av = emit_attn_chunk(B - 1, qc, *tiles)
                agq.append(("c", B - 1, qc))
                drain_op(8, last_av, tail=True)
            drain_op(0, last_av, tail=True)

    nc.compile()
    return nc


def shard_inputs(x, rope_cos, rope_sin, W_qkv, W_out, n_cores):
    """Host-side prep: transpose x, build rope tables in [d, pos] layout with
    the rotation sign folded in, slice per-core weight shards, cast to bf16."""
    B, T, D = x.shape
    H = W_qkv.shape[1] // (3 * P)
    HPC = H // n_cores
    NOUT = W_out.shape[1] // n_cores
    bf = ml_dtypes.bfloat16

    xT = np.ascontiguousarray(x.reshape(B * T, D).T).astype(bf)
    cosT = np.ascontiguousarray(rope_cos.T).astype(bf)          # [P, T]
    sinT = rope_sin.T.copy()
    sinT[:P // 2] = -sinT[:P // 2]
    sinT = np.ascontiguousarray(sinT).astype(bf)

    Wq3 = W_qkv.reshape(D, 3, H, P)  # [D, qkv, head, d]
    in_maps = []
    for c in range(n_cores):
        heads = range(c * HPC, (c + 1) * HPC)
        cols = [Wq3[:, i, h, :] for i in range(3) for h in heads]
        wqkv_c = np.ascontiguousarray(
            np.concatenate(cols, axis=1)).astype(bf)            # [D, 3*HPC*P]
        wout_c = np.ascontiguousarray(
            W_out[:, c * NOUT:(c + 1) * NOUT]).astype(bf)
        in_maps.append({
            "xT": xT, "wqkv": wqkv_c, "wout": wout_c,
            "ropec": cosT, "ropes": sinT,
        })
    return in_maps


def assemble_output(results, B, T, D, n_cores):
    NOUT = D // n_cores
    out = np.empty((B * T, D), np.float32)
    for c in range(n_cores):
        # device emits the transposed projection [oc, tok]; the last
        # batch's first q-chunk comes separately in [tok, oc]
        out[:, c * NOUT:(c + 1) * NOUT] = results[c]["out"].T
        out[(B - 1) * T:(B - 1) * T + CH, c * NOUT:(c + 1) * NOUT] = \
            results[c]["outS"]
    return out.reshape(B, T, D)


_NC_CACHE = {}


def _get_nc(B, T, D, H, n_cores):
    key = (B, T, D, H, n_cores)
    if key not in _NC_CACHE:
        _NC_CACHE[key] = build_nc(B, T, D, H, n_cores)
    return _NC_CACHE[key]


def run(x, rope_cos, rope_sin, W_qkv, W_out, trace=False):
    B, T, D = x.shape
    H = W_qkv.shape[1] // (3 * P)
    n_cores = N_CORES
    nc = _get_nc(B, T, D, H, n_cores)
    in_maps = shard_inputs(np.asarray(x, np.float32),
                           np.asarray(rope_cos, np.float32),
                           np.asarray(rope_sin, np.float32),
                           np.asarray(W_qkv, np.float32),
                           np.asarray(W_out, np.float32), n_cores)
    res = run_bass_kernel_spmd(nc, in_maps, core_ids=list(range(n_cores)),
                               trace=trace)
    out = assemble_output(res.results, B, T, D, n_cores)
    return out, res


def kernel(x, rope_cos, rope_sin, W_qkv, W_out):
    out, _ = run(x, rope_cos, rope_sin, W_qkv, W_out, trace=False)
    return out
